# revision 1
# baseline (speedup 1.0000x reference)
"""TRN2 Bass kernel for causal multi-head attention with RoPE.

Problem: B=2, S=2048, HID=2048, NH=16, HD=128 (fp32).
Sharding: 8 cores = 2 (batch) x 4 (head-groups of 4 heads).
Each core computes q/k/v projections for its 4 heads (column-parallel),
RoPE, causal attention, and a row-parallel partial o_proj; the host sums
the 4 partials per batch.

Per-core device program (matmuls fp32r at full PE rate; P@V in bf16):
  Phase P: V = x @ Wv_g (natural layout, bf16 + ones column),
           QT/KT = (W x)^T with RoPE fused into the PSUM eviction
           (ACT copies PSUM->SBUF, DVE does full-width rotate ops),
           QT/KT spilled to DRAM (SBUF pressure).
  Phase A: chunk-outer loop; per (chunk, head): scores^T = KT^T Q with
           causal tile skipping and N-trimmed diagonal tiles, exp on ACT
           (scale fused), bf16 0/1 causal mask multiplied on GPSIMD,
           P@[V|1] accumulated in PSUM (ones column = softmax sums),
           per-partition normalize, PE transpose into attn_outT.
           o_proj for the finished seq-tiles is interleaved per chunk to
           fill PE dependency stalls.
"""
import os
import sys

if "/opt/trn_rl_repo" not in sys.path:
    sys.path.insert(0, "/opt/trn_rl_repo")

import numpy as np
import ml_dtypes

import concourse.bass as bass
import concourse.mybir as mybir
import concourse.tile as tile
from concourse import bacc
from concourse.bass_utils import run_bass_kernel_spmd
from concourse.masks import make_identity
from contextlib import ExitStack

P = 128
B, S, HID, NH = 2, 2048, 2048, 16
HD = HID // NH              # 128
H = 4                       # heads per core
DPC = H * HD                # 512 dims per core
KO = HID // P               # 16 contraction chunks
SC = S // 512               # 4 seq chunks of 512
ST = S // P                 # 16 seq tiles of 128
SCALE = 1.0 / float(np.sqrt(HD))

f32 = mybir.dt.float32
f32r = mybir.dt.float32r
bf16 = mybir.dt.bfloat16

_CACHED_NC = None


def build_nc():
    AF = mybir.ActivationFunctionType
    nc = bacc.Bacc(None, target_bir_lowering=False)

    xt = nc.declare_dram_parameter("xt", [P, KO, S], f32r, isOutput=False)
    wq = nc.declare_dram_parameter("wq", [H, P, KO, HD], f32r, isOutput=False)
    wk = nc.declare_dram_parameter("wk", [H, P, KO, HD], f32r, isOutput=False)
    wv = nc.declare_dram_parameter("wv", [P, KO, DPC], f32r, isOutput=False)
    wo = nc.declare_dram_parameter("wo", [P, H, HID], f32r, isOutput=False)
    cosf = nc.declare_dram_parameter("cosf", [P, S], f32, isOutput=False)
    sinf = nc.declare_dram_parameter("sinf", [P, S], f32, isOutput=False)
    bmask = nc.declare_dram_parameter("bmask", [P, H, 512], bf16, isOutput=False)
    out_p = nc.declare_dram_parameter("out_p", [S, HID], f32, isOutput=True)

    out3 = out_p.rearrange("(st p) n -> p st n", p=P)

    with tile.TileContext(nc) as tc:
        with ExitStack() as top:
            vpool = top.enter_context(tc.tile_pool(name="vpool", bufs=1))
            const = top.enter_context(tc.tile_pool(name="const", bufs=1))
            dram = top.enter_context(tc.tile_pool(name="dram", bufs=1, space="DRAM"))

            vsb = vpool.tile([P, ST, H, 132], bf16)
            nc.vector.memset(vsb[:, :, :, 128:132], 1.0)

            qt_sp = dram.tile([H, P, S], f32r)
            kt_sp = dram.tile([H, P, S], f32r)

            # ---------------- Phase P: projections ----------------
            with ExitStack() as ctx:
                xpool = ctx.enter_context(tc.tile_pool(name="xp", bufs=1))
                pp = ctx.enter_context(tc.tile_pool(name="pp", bufs=4, space="PSUM"))

                # x load interleaved seq-chunk-major so V matmuls can start
                # after the first chunk; alternate the two HWDGE queues.
                # wv half 0 is issued before x so the first V matmuls are not
                # starved behind the 16MB x stream.
                xs = xpool.tile([P, KO, S], f32r)
                # q/k weight pool opens early (bufs=1, 8KB) so the first
                # wq tile prefetches during the V phase
                wpool = ctx.enter_context(tc.tile_pool(name="wqk", bufs=1))
                with tc.tile_pool(name="wvp", bufs=2) as wvp:
                    wvts = [
                        wvp.tile([P, KO, DPC // 2], f32r, tag="wv", name=f"wv{vh}")
                        for vh in range(2)
                    ]
                    nc.sync.dma_start(wvts[0][:, :, 0:128], wv[:, :, 0:128])
                    nc.scalar.dma_start(wvts[0][:, :, 128:256], wv[:, :, 128:256])
                    for sc in range(SC):
                        for ko in range(KO):
                            eng = nc.sync if (ko % 2 == 0) else nc.scalar
                            eng.dma_start(xs[:, ko, sc * 512:(sc + 1) * 512],
                                          xt[:, ko, sc * 512:(sc + 1) * 512])
                        if sc == 0:
                            nc.sync.dma_start(wvts[1][:, :, 0:128], wv[:, :, 256:384])
                            nc.scalar.dma_start(wvts[1][:, :, 128:256], wv[:, :, 384:512])

                    for vh in range(2):
                        wvt = wvts[vh]
                        for st in range(ST):
                            ps = pp.tile([P, 256], f32, tag="vproj")
                            for ko in range(KO):
                                nc.tensor.matmul(
                                    ps[:],
                                    xs[:, ko, st * P:(st + 1) * P],
                                    wvt[:, ko],
                                    start=(ko == 0),
                                    stop=(ko == KO - 1),
                                )
                            nc.vector.tensor_copy(
                                vsb[:, st, vh * 2:(vh + 1) * 2, 0:128],
                                ps.rearrange("p (h d) -> p h d", h=2),
                            )

                cspool = ctx.enter_context(tc.tile_pool(name="cs", bufs=1))
                rtmp = ctx.enter_context(tc.tile_pool(name="rt", bufs=2))
                spill = ctx.enter_context(tc.tile_pool(name="sp", bufs=2))
                # full-height tables: cos duplicated halves; sin signed
                # (-sin rows 0:64, +sin rows 64:128) so the combine is one add
                cosT = cspool.tile([P, S], f32)
                sinT = cspool.tile([P, S], f32)
                nc.scalar.dma_start(cosT[:], cosf[:])
                nc.scalar.dma_start(sinT[:], sinf[:])

                if True:
                    for w4, sp_dram in ((wq, qt_sp), (wk, kt_sp)):
                        for h in range(H):
                            wt = wpool.tile([P, KO, HD], f32r, tag="w")
                            nc.sync.dma_start(wt[:], w4[h])
                            for sc in range(SC):
                                ssl = slice(sc * 512, (sc + 1) * 512)
                                ps = pp.tile([P, 512], f32, tag="proj")
                                for ko in range(KO):
                                    nc.tensor.matmul(
                                        ps[:],
                                        wt[:, ko],
                                        xs[:, ko, ssl],
                                        start=(ko == 0),
                                        stop=(ko == KO - 1),
                                    )
                                # RoPE eviction: partition-shifted reads are
                                # legal only with a PSUM operand, so the two
                                # rotate half-ops read ps directly; the combine
                                # runs full-width on SBUF.
                                t0 = rtmp.tile([P, 512], f32, tag="t0")
                                spt = spill.tile([P, 512], f32r, tag="spl")
                                nc.vector.tensor_mul(t0[0:64], ps[64:128], sinT[0:64, ssl])
                                nc.vector.tensor_mul(t0[64:128], ps[0:64], sinT[64:128, ssl])
                                nc.vector.tensor_mul(spt[:], ps[:], cosT[:, ssl])
                                nc.vector.tensor_add(spt[:], spt[:], t0[:])
                                nc.gpsimd.dma_start(sp_dram[h][:, ssl], spt[:])

            # ------------- Phase A: attention + interleaved o_proj -------------
            with ExitStack() as ctx:
                kpool = ctx.enter_context(tc.tile_pool(name="kp", bufs=1))
                qcpool = ctx.enter_context(tc.tile_pool(name="qc", bufs=2))
                ppool = ctx.enter_context(tc.tile_pool(name="ppool", bufs=6))
                stage = ctx.enter_context(tc.tile_pool(name="stage", bufs=4))
                aopool = ctx.enter_context(tc.tile_pool(name="ao", bufs=1))
                wopool = ctx.enter_context(tc.tile_pool(name="wop", bufs=1))
                ost = ctx.enter_context(tc.tile_pool(name="ost", bufs=4))
                spsum = ctx.enter_context(tc.tile_pool(name="sps", bufs=2, space="PSUM"))
                opsum = ctx.enter_context(tc.tile_pool(name="ops", bufs=2, space="PSUM"))
                opo = ctx.enter_context(tc.tile_pool(name="opo", bufs=2, space="PSUM"))

                bmt = const.tile([P, H, 512], bf16)
                nc.scalar.dma_start(bmt[:], bmask[:])
                zb = const.tile([P, 1], f32)
                nc.vector.memset(zb[:], 0.0)
                ones_col = const.tile([P, 1], bf16)
                nc.vector.memset(ones_col[:], 1.0)

                kall = kpool.tile([P, H, S], f32r)

                aot_c = [
                    aopool.tile([P, H, 512], f32r, tag=f"aot{c}", name=f"aot{c}")
                    for c in range(SC)
                ]

                def emit_oproj(cc):
                    for st4 in range(4):
                        st = cc * 4 + st4
                        for nch in range(4):
                            pso = opo.tile([P, 512], f32, tag="po", name="pso")
                            for dc in range(H):
                                nc.tensor.matmul(
                                    pso[:],
                                    aot_c[cc][:, dc, st4 * P:(st4 + 1) * P],
                                    wot[:, dc, nch * 512:(nch + 1) * 512],
                                    start=(dc == 0),
                                    stop=(dc == H - 1),
                                )
                            ob = ost.tile([P, 512], f32, tag="ob", name="ob")
                            nc.vector.tensor_copy(ob[:], pso[:])
                            nc.sync.dma_start(
                                out3[:, st, nch * 512:(nch + 1) * 512], ob[:]
                            )

                qcs = []
                for c in range(SC):
                    qc = qcpool.tile([P, H, 512], f32r, tag="qc", name=f"qc{c}")
                    qcs.append(qc)
                # reloads are sliced per 512-chunk and ordered chunk-major so
                # the first scores are gated on ~512KB, not the full 6MB
                for h in range(H):
                    eng = nc.scalar if h % 2 == 0 else nc.sync
                    eng.dma_start(qcs[0][:, h], qt_sp[h][:, 0:512])
                for cc in range(SC):
                    for h in range(H):
                        eng = nc.scalar if (cc * H + h) % 2 == 0 else nc.sync
                        eng.dma_start(kall[:, h, cc * 512:(cc + 1) * 512],
                                      kt_sp[h][:, cc * 512:(cc + 1) * 512])
                    if cc == 1:
                        wot = wopool.tile([P, H, HID], f32r)
                        nc.sync.dma_start(wot[:], wo[:])

                for c in range(SC):
                    qc = qcs[c]
                    if c > 0:
                        for h in range(H):
                            eng = nc.scalar if h % 2 == 0 else nc.sync
                            eng.dma_start(qc[:, h], qt_sp[h][:, c * 512:(c + 1) * 512])
                    nt = 4 * (c + 1)
                    for h in range(H):
                        # attn_outT accumulator [d, sq] and softmax sums [1, sq]
                        ob_ps = opsum.tile([P, 512], f32, tag="obp", name="obp")
                        sm_ps = opsum.tile([1, 512], f32, tag="smp", name="smp")
                        # diagonal tiles first: their exp+mask latency hides
                        # behind the dense unmasked tail of this head and the
                        # previous head's stream
                        t_order = list(range(4 * c, nt)) + list(range(0, 4 * c))
                        for ti, t in enumerate(t_order):
                            r = t - 4 * c
                            off = P * max(r, 0)
                            ps = spsum.tile([P, 512], f32, tag="s")
                            nc.tensor.matmul(
                                ps[:, off:512],
                                kall[:, h, t * P:(t + 1) * P],
                                qc[:, h, off:512],
                                start=True,
                                stop=True,
                            )
                            pt = ppool.tile([P, 512], bf16, tag="pt")
                            nc.scalar.activation(
                                pt[:, off:512], ps[:, off:512], AF.Exp,
                                bias=zb[:], scale=SCALE,
                            )
                            if r >= 0:
                                nc.vector.tensor_mul(
                                    pt[:, off:512], pt[:, off:512], bmt[:, r, off:512]
                                )
                            # P@V with V stationary (one LDWEIGHTS per tile);
                            # output is attn_outT [d, sq] directly
                            nc.tensor.matmul(
                                ob_ps[:, off:512],
                                vsb[:, t, h, 0:128],
                                pt[:, off:512],
                                start=(ti == 0),
                                stop=(ti == nt - 1),
                            )
                            nc.tensor.matmul(
                                sm_ps[:, off:512],
                                ones_col[:],
                                pt[:, off:512],
                                start=(ti == 0),
                                stop=(ti == nt - 1),
                            )
                        # normalize: rcp -> PE broadcast to 128 partitions ->
                        # single DVE multiply into attn_outT SBUF
                        rcp = stage.tile([1, 512], f32, tag="rcp")
                        nc.vector.reciprocal_approx_fast(rcp[:], sm_ps[:])
                        bc_sb = stage.tile([P, 512], f32, tag="bc")
                        nc.gpsimd.partition_broadcast(bc_sb[:], rcp[:])
                        nc.vector.tensor_mul(aot_c[c][:, h], ob_ps[:], bc_sb[:])

                    # o_proj deferred by one chunk: its aot inputs are then
                    # guaranteed ready, so the PE stream never stalls on the
                    # normalize tail
                    if c > 0:
                        emit_oproj(c - 1)
                emit_oproj(SC - 1)

    nc.compile()
    return nc


def _host_prep(hidden_states, position_ids, Wq, Wk, Wv, Wo):
    """Build the 8 per-core input maps."""
    inv_freq = 1.0 / (10000.0 ** (np.arange(0, HD, 2, dtype=np.float32) / HD))
    t = np.arange(S, dtype=np.float32)
    freqs = np.outer(t, inv_freq).astype(np.float32)  # [S, 64]

    bm = np.empty((P, H, 512), dtype=np.float32)
    i = np.arange(P)[:, None, None]
    r = np.arange(H)[None, :, None]
    j = np.arange(512)[None, None, :]
    bm[:] = np.where(i + P * r <= j, 1.0, 0.0)
    bm = bm.astype(ml_dtypes.bfloat16)

    in_maps = []
    per_batch = []
    for b in range(B):
        xT = np.ascontiguousarray(hidden_states[b].T)  # [HID, S]
        xt_sw = np.ascontiguousarray(
            xT.reshape(KO, P, S).transpose(1, 0, 2)
        )  # [P, KO, S]
        fp = freqs[position_ids[b]]  # [S, 64]
        ch = np.cos(fp).T            # [64, S]
        sh = np.sin(fp).T
        cosf = np.ascontiguousarray(np.concatenate([ch, ch], axis=0))   # [128, S]
        sinf = np.ascontiguousarray(np.concatenate([-sh, sh], axis=0))  # signed
        per_batch.append((xt_sw, cosf, sinf))

    for core in range(8):
        b, hg = core // 4, core % 4
        sl = slice(hg * DPC, (hg + 1) * DPC)
        xt_sw, cosf, sinf = per_batch[b]
        wq_sw = np.ascontiguousarray(
            Wq[sl].T.reshape(KO, P, H, HD).transpose(2, 1, 0, 3)
        )  # [H, P, KO, HD]
        wk_sw = np.ascontiguousarray(
            Wk[sl].T.reshape(KO, P, H, HD).transpose(2, 1, 0, 3)
        )
        wv_sw = np.ascontiguousarray(
            Wv[sl].T.reshape(KO, P, DPC).transpose(1, 0, 2)
        )  # [P, KO, DPC]
        wo_sw = np.ascontiguousarray(
            Wo[:, sl].T.reshape(H, HD, HID).transpose(1, 0, 2)
        )  # [P, H, HID]
        in_maps.append({
            "xt": xt_sw, "wq": wq_sw, "wk": wk_sw, "wv": wv_sw, "wo": wo_sw,
            "cosf": cosf, "sinf": sinf, "bmask": bm,
        })
    return in_maps


def kernel(hidden_states, attention_mask, position_ids, Wq, Wk, Wv, Wo,
           _trace=False, _trace_kwargs=None):
    global _CACHED_NC
    hidden_states = np.asarray(hidden_states, dtype=np.float32)
    position_ids = np.asarray(position_ids)
    Wq, Wk, Wv, Wo = (np.asarray(w, dtype=np.float32) for w in (Wq, Wk, Wv, Wo))

    if _CACHED_NC is None:
        _CACHED_NC = build_nc()
    nc = _CACHED_NC

    in_maps = _host_prep(hidden_states, position_ids, Wq, Wk, Wv, Wo)
    res = run_bass_kernel_spmd(
        nc, in_maps, list(range(8)), trace=_trace, **(_trace_kwargs or {})
    )

    out = np.empty((B, S, HID), dtype=np.float32)
    for b in range(B):
        acc = res.results[b * 4]["out_p"].astype(np.float32)
        for hg in range(1, 4):
            acc = acc + res.results[b * 4 + hg]["out_p"]
        out[b] = acc
    if _trace:
        return out, res
    return out



# revision 2
# speedup vs baseline: 1.1889x; 1.1889x over previous
"""TRN2 Bass kernel for causal multi-head attention with RoPE.

Problem: B=2, S=2048, HID=2048, NH=16, HD=128 (fp32).
Sharding: 8 cores = 2 (batch) x 4 (head-groups of 4 heads).
Each core computes q/k/v projections for its 4 heads (column-parallel),
RoPE, causal attention, and a row-parallel partial o_proj; the host sums
the 4 partials per batch.

Per-core device program (matmuls fp32r at full PE rate; P@V in bf16):
  Phase P: V = x @ Wv_g (natural layout, bf16 + ones column),
           QT/KT = (W x)^T with RoPE fused into the PSUM eviction
           (ACT copies PSUM->SBUF, DVE does full-width rotate ops),
           QT/KT spilled to DRAM (SBUF pressure).
  Phase A: chunk-outer loop; per (chunk, head): scores^T = KT^T Q with
           causal tile skipping and N-trimmed diagonal tiles, exp on ACT
           (scale fused), bf16 0/1 causal mask multiplied on GPSIMD,
           P@[V|1] accumulated in PSUM (ones column = softmax sums),
           per-partition normalize, PE transpose into attn_outT.
           o_proj for the finished seq-tiles is interleaved per chunk to
           fill PE dependency stalls.
"""
import os
import sys

if "/opt/trn_rl_repo" not in sys.path:
    sys.path.insert(0, "/opt/trn_rl_repo")

import numpy as np
import ml_dtypes

import concourse.bass as bass
import concourse.mybir as mybir
import concourse.tile as tile
from concourse import bacc
from concourse.bass_utils import run_bass_kernel_spmd
from concourse.masks import make_identity
from contextlib import ExitStack

P = 128
B, S, HID, NH = 2, 2048, 2048, 16
HD = HID // NH              # 128
H = 4                       # heads per core
DPC = H * HD                # 512 dims per core
KO = HID // P               # 16 contraction chunks
SC = S // 512               # 4 seq chunks of 512
ST = S // P                 # 16 seq tiles of 128
SCALE = 1.0 / float(np.sqrt(HD))

f32 = mybir.dt.float32
f32r = mybir.dt.float32r
bf16 = mybir.dt.bfloat16

_CACHED_NC = None


def build_nc():
    AF = mybir.ActivationFunctionType
    nc = bacc.Bacc(None, target_bir_lowering=False)

    xt = nc.declare_dram_parameter("xt", [P, KO, S], f32r, isOutput=False)
    wq = nc.declare_dram_parameter("wq", [H, P, KO, HD], f32r, isOutput=False)
    wk = nc.declare_dram_parameter("wk", [H, P, KO, HD], f32r, isOutput=False)
    wv = nc.declare_dram_parameter("wv", [P, KO, DPC], f32r, isOutput=False)
    wo = nc.declare_dram_parameter("wo", [P, H, HID], f32r, isOutput=False)
    cosf = nc.declare_dram_parameter("cosf", [P, S], f32, isOutput=False)
    sinf = nc.declare_dram_parameter("sinf", [P, S], f32, isOutput=False)
    bmask = nc.declare_dram_parameter("bmask", [P, H, 512], bf16, isOutput=False)
    out_p = nc.declare_dram_parameter("out_p", [S, HID], f32, isOutput=True)

    out3 = out_p.rearrange("(st p) n -> p st n", p=P)

    with tile.TileContext(nc) as tc:
        with ExitStack() as top:
            vpool = top.enter_context(tc.tile_pool(name="vpool", bufs=1))
            const = top.enter_context(tc.tile_pool(name="const", bufs=1))
            dram = top.enter_context(tc.tile_pool(name="dram", bufs=1, space="DRAM"))

            vsb = vpool.tile([P, ST, H, 132], bf16)
            nc.vector.memset(vsb[:, :, :, 128:132], 1.0)

            qt_sp = dram.tile([H, P, S], f32r)
            kt_sp = dram.tile([H, P, S], f32r)

            # ---------------- Phase P: projections ----------------
            with ExitStack() as ctx:
                xpool = ctx.enter_context(tc.tile_pool(name="xp", bufs=1))
                pp = ctx.enter_context(tc.tile_pool(name="pp", bufs=4, space="PSUM"))

                # x load interleaved seq-chunk-major so V matmuls can start
                # after the first chunk; alternate the two HWDGE queues.
                # wv half 0 is issued before x so the first V matmuls are not
                # starved behind the 16MB x stream.
                xs = xpool.tile([P, KO, S], f32r)
                # q/k weight pool opens early so the first wq tile prefetches
                # during the V phase; bufs=2 so each head's 1MB weight load
                # overlaps the previous head's matmuls (PE never stalls)
                wpool = ctx.enter_context(tc.tile_pool(name="wqk", bufs=2))
                with tc.tile_pool(name="wvp", bufs=2) as wvp:
                    wvts = [
                        wvp.tile([P, KO, DPC // 2], f32r, tag="wv", name=f"wv{vh}")
                        for vh in range(2)
                    ]
                    nc.sync.dma_start(wvts[0][:, :, 0:128], wv[:, :, 0:128])
                    nc.scalar.dma_start(wvts[0][:, :, 128:256], wv[:, :, 128:256])
                    for sc in range(SC):
                        for ko in range(KO):
                            eng = nc.sync if (ko % 2 == 0) else nc.scalar
                            eng.dma_start(xs[:, ko, sc * 512:(sc + 1) * 512],
                                          xt[:, ko, sc * 512:(sc + 1) * 512])
                        if sc == 0:
                            nc.sync.dma_start(wvts[1][:, :, 0:128], wv[:, :, 256:384])
                            nc.scalar.dma_start(wvts[1][:, :, 128:256], wv[:, :, 384:512])

                    for vh in range(2):
                        wvt = wvts[vh]
                        for st in range(ST):
                            ps = pp.tile([P, 256], f32, tag="vproj")
                            for ko in range(KO):
                                nc.tensor.matmul(
                                    ps[:],
                                    xs[:, ko, st * P:(st + 1) * P],
                                    wvt[:, ko],
                                    start=(ko == 0),
                                    stop=(ko == KO - 1),
                                )
                            nc.vector.tensor_copy(
                                vsb[:, st, vh * 2:(vh + 1) * 2, 0:128],
                                ps.rearrange("p (h d) -> p h d", h=2),
                            )

                cspool = ctx.enter_context(tc.tile_pool(name="cs", bufs=1))
                rtmp = ctx.enter_context(tc.tile_pool(name="rt", bufs=2))
                spill = ctx.enter_context(tc.tile_pool(name="sp", bufs=2))
                # full-height tables: cos duplicated halves; sin signed
                # (-sin rows 0:64, +sin rows 64:128) so the combine is one add
                cosT = cspool.tile([P, S], f32)
                sinT = cspool.tile([P, S], f32)
                nc.scalar.dma_start(cosT[:], cosf[:])
                nc.scalar.dma_start(sinT[:], sinf[:])

                if True:
                    for w4, sp_dram in ((wq, qt_sp), (wk, kt_sp)):
                        for h in range(H):
                            wt = wpool.tile([P, KO, HD], f32r, tag="w")
                            nc.sync.dma_start(wt[:], w4[h])
                            for sc in range(SC):
                                ssl = slice(sc * 512, (sc + 1) * 512)
                                ps = pp.tile([P, 512], f32, tag="proj")
                                for ko in range(KO):
                                    nc.tensor.matmul(
                                        ps[:],
                                        wt[:, ko],
                                        xs[:, ko, ssl],
                                        start=(ko == 0),
                                        stop=(ko == KO - 1),
                                    )
                                # RoPE eviction: partition-shifted reads are
                                # legal only with a PSUM operand, so the two
                                # rotate half-ops read ps directly; the combine
                                # runs full-width on SBUF.
                                t0 = rtmp.tile([P, 512], f32, tag="t0")
                                spt = spill.tile([P, 512], f32r, tag="spl")
                                nc.vector.tensor_mul(t0[0:64], ps[64:128], sinT[0:64, ssl])
                                nc.vector.tensor_mul(t0[64:128], ps[0:64], sinT[64:128, ssl])
                                nc.vector.tensor_mul(spt[:], ps[:], cosT[:, ssl])
                                nc.vector.tensor_add(spt[:], spt[:], t0[:])
                                nc.gpsimd.dma_start(sp_dram[h][:, ssl], spt[:])

            # ------------- Phase A: attention + interleaved o_proj -------------
            with ExitStack() as ctx:
                kpool = ctx.enter_context(tc.tile_pool(name="kp", bufs=1))
                qcpool = ctx.enter_context(tc.tile_pool(name="qc", bufs=2))
                ppool = ctx.enter_context(tc.tile_pool(name="ppool", bufs=6))
                stage = ctx.enter_context(tc.tile_pool(name="stage", bufs=4))
                aopool = ctx.enter_context(tc.tile_pool(name="ao", bufs=1))
                wopool = ctx.enter_context(tc.tile_pool(name="wop", bufs=1))
                ost = ctx.enter_context(tc.tile_pool(name="ost", bufs=4))
                spsum = ctx.enter_context(tc.tile_pool(name="sps", bufs=2, space="PSUM"))
                opsum = ctx.enter_context(tc.tile_pool(name="ops", bufs=2, space="PSUM"))
                opo = ctx.enter_context(tc.tile_pool(name="opo", bufs=2, space="PSUM"))

                bmt = const.tile([P, H, 512], bf16)
                nc.scalar.dma_start(bmt[:], bmask[:])
                zb = const.tile([P, 1], f32)
                nc.vector.memset(zb[:], 0.0)
                ones_col = const.tile([P, 1], bf16)
                nc.vector.memset(ones_col[:], 1.0)

                kall = kpool.tile([P, H, S], f32r)

                aot_c = [
                    aopool.tile([P, H, 512], f32r, tag=f"aot{c}", name=f"aot{c}")
                    for c in range(SC)
                ]

                def emit_oproj(cc):
                    for st4 in range(4):
                        st = cc * 4 + st4
                        for nch in range(4):
                            pso = opo.tile([P, 512], f32, tag="po", name="pso")
                            for dc in range(H):
                                nc.tensor.matmul(
                                    pso[:],
                                    aot_c[cc][:, dc, st4 * P:(st4 + 1) * P],
                                    wot[:, dc, nch * 512:(nch + 1) * 512],
                                    start=(dc == 0),
                                    stop=(dc == H - 1),
                                )
                            ob = ost.tile([P, 512], f32, tag="ob", name="ob")
                            nc.vector.tensor_copy(ob[:], pso[:])
                            nc.sync.dma_start(
                                out3[:, st, nch * 512:(nch + 1) * 512], ob[:]
                            )

                qcs = []
                for c in range(SC):
                    qc = qcpool.tile([P, H, 512], f32r, tag="qc", name=f"qc{c}")
                    qcs.append(qc)
                # reloads are sliced per 512-chunk and ordered chunk-major so
                # the first scores are gated on ~512KB, not the full 6MB
                for h in range(H):
                    eng = nc.scalar if h % 2 == 0 else nc.sync
                    eng.dma_start(qcs[0][:, h], qt_sp[h][:, 0:512])
                for cc in range(SC):
                    for h in range(H):
                        eng = nc.scalar if (cc * H + h) % 2 == 0 else nc.sync
                        eng.dma_start(kall[:, h, cc * 512:(cc + 1) * 512],
                                      kt_sp[h][:, cc * 512:(cc + 1) * 512])
                    if cc == 1:
                        wot = wopool.tile([P, H, HID], f32r)
                        nc.sync.dma_start(wot[:], wo[:])

                for c in range(SC):
                    qc = qcs[c]
                    if c > 0:
                        for h in range(H):
                            eng = nc.scalar if h % 2 == 0 else nc.sync
                            eng.dma_start(qc[:, h], qt_sp[h][:, c * 512:(c + 1) * 512])
                    nt = 4 * (c + 1)
                    for h in range(H):
                        # attn_outT accumulator [d, sq] and softmax sums [1, sq]
                        ob_ps = opsum.tile([P, 512], f32, tag="obp", name="obp")
                        sm_ps = opsum.tile([1, 512], f32, tag="smp", name="smp")
                        # diagonal tiles first: their exp+mask latency hides
                        # behind the dense unmasked tail of this head and the
                        # previous head's stream
                        t_order = list(range(4 * c, nt)) + list(range(0, 4 * c))
                        for ti, t in enumerate(t_order):
                            r = t - 4 * c
                            off = P * max(r, 0)
                            ps = spsum.tile([P, 512], f32, tag="s")
                            nc.tensor.matmul(
                                ps[:, off:512],
                                kall[:, h, t * P:(t + 1) * P],
                                qc[:, h, off:512],
                                start=True,
                                stop=True,
                            )
                            pt = ppool.tile([P, 512], bf16, tag="pt")
                            nc.scalar.activation(
                                pt[:, off:512], ps[:, off:512], AF.Exp,
                                bias=zb[:], scale=SCALE,
                            )
                            if r >= 0:
                                nc.vector.tensor_mul(
                                    pt[:, off:512], pt[:, off:512], bmt[:, r, off:512]
                                )
                            # P@V with V stationary (one LDWEIGHTS per tile);
                            # output is attn_outT [d, sq] directly
                            nc.tensor.matmul(
                                ob_ps[:, off:512],
                                vsb[:, t, h, 0:128],
                                pt[:, off:512],
                                start=(ti == 0),
                                stop=(ti == nt - 1),
                            )
                            nc.tensor.matmul(
                                sm_ps[:, off:512],
                                ones_col[:],
                                pt[:, off:512],
                                start=(ti == 0),
                                stop=(ti == nt - 1),
                            )
                        # normalize: rcp -> PE broadcast to 128 partitions ->
                        # single DVE multiply into attn_outT SBUF
                        rcp = stage.tile([1, 512], f32, tag="rcp")
                        nc.vector.reciprocal_approx_fast(rcp[:], sm_ps[:])
                        bc_sb = stage.tile([P, 512], f32, tag="bc")
                        nc.gpsimd.partition_broadcast(bc_sb[:], rcp[:])
                        nc.vector.tensor_mul(aot_c[c][:, h], ob_ps[:], bc_sb[:])

                    # o_proj deferred by one chunk: its aot inputs are then
                    # guaranteed ready, so the PE stream never stalls on the
                    # normalize tail
                    if c > 0:
                        emit_oproj(c - 1)
                emit_oproj(SC - 1)

    nc.compile()
    return nc


def _host_prep(hidden_states, position_ids, Wq, Wk, Wv, Wo):
    """Build the 8 per-core input maps."""
    inv_freq = 1.0 / (10000.0 ** (np.arange(0, HD, 2, dtype=np.float32) / HD))
    t = np.arange(S, dtype=np.float32)
    freqs = np.outer(t, inv_freq).astype(np.float32)  # [S, 64]

    bm = np.empty((P, H, 512), dtype=np.float32)
    i = np.arange(P)[:, None, None]
    r = np.arange(H)[None, :, None]
    j = np.arange(512)[None, None, :]
    bm[:] = np.where(i + P * r <= j, 1.0, 0.0)
    bm = bm.astype(ml_dtypes.bfloat16)

    in_maps = []
    per_batch = []
    for b in range(B):
        xT = np.ascontiguousarray(hidden_states[b].T)  # [HID, S]
        xt_sw = np.ascontiguousarray(
            xT.reshape(KO, P, S).transpose(1, 0, 2)
        )  # [P, KO, S]
        fp = freqs[position_ids[b]]  # [S, 64]
        ch = np.cos(fp).T            # [64, S]
        sh = np.sin(fp).T
        cosf = np.ascontiguousarray(np.concatenate([ch, ch], axis=0))   # [128, S]
        sinf = np.ascontiguousarray(np.concatenate([-sh, sh], axis=0))  # signed
        per_batch.append((xt_sw, cosf, sinf))

    for core in range(8):
        b, hg = core // 4, core % 4
        sl = slice(hg * DPC, (hg + 1) * DPC)
        xt_sw, cosf, sinf = per_batch[b]
        wq_sw = np.ascontiguousarray(
            Wq[sl].T.reshape(KO, P, H, HD).transpose(2, 1, 0, 3)
        )  # [H, P, KO, HD]
        wk_sw = np.ascontiguousarray(
            Wk[sl].T.reshape(KO, P, H, HD).transpose(2, 1, 0, 3)
        )
        wv_sw = np.ascontiguousarray(
            Wv[sl].T.reshape(KO, P, DPC).transpose(1, 0, 2)
        )  # [P, KO, DPC]
        wo_sw = np.ascontiguousarray(
            Wo[:, sl].T.reshape(H, HD, HID).transpose(1, 0, 2)
        )  # [P, H, HID]
        in_maps.append({
            "xt": xt_sw, "wq": wq_sw, "wk": wk_sw, "wv": wv_sw, "wo": wo_sw,
            "cosf": cosf, "sinf": sinf, "bmask": bm,
        })
    return in_maps


def kernel(hidden_states, attention_mask, position_ids, Wq, Wk, Wv, Wo,
           _trace=False, _trace_kwargs=None):
    global _CACHED_NC
    hidden_states = np.asarray(hidden_states, dtype=np.float32)
    position_ids = np.asarray(position_ids)
    Wq, Wk, Wv, Wo = (np.asarray(w, dtype=np.float32) for w in (Wq, Wk, Wv, Wo))

    if _CACHED_NC is None:
        _CACHED_NC = build_nc()
    nc = _CACHED_NC

    in_maps = _host_prep(hidden_states, position_ids, Wq, Wk, Wv, Wo)
    res = run_bass_kernel_spmd(
        nc, in_maps, list(range(8)), trace=_trace, **(_trace_kwargs or {})
    )

    out = np.empty((B, S, HID), dtype=np.float32)
    for b in range(B):
        acc = res.results[b * 4]["out_p"].astype(np.float32)
        for hg in range(1, 4):
            acc = acc + res.results[b * 4 + hg]["out_p"]
        out[b] = acc
    if _trace:
        return out, res
    return out



# revision 3
# speedup vs baseline: 1.3998x; 1.1774x over previous
"""TRN2 Bass kernel for causal multi-head attention with RoPE (v2).

Problem: B=2, S=2048, HID=2048, NH=16, HD=128 (fp32 reference).
Sharding: 8 cores = 2 (batch) x 4 (head-groups of 4 heads).
Each core computes q/k/v projections for its 4 heads (column-parallel),
RoPE, causal attention, and a row-parallel partial o_proj; the host sums
the 4 partials per batch.

v2 design (vs v1): all matmul operands in bf16 (PSUM accumulates fp32),
no DRAM spill of Q/K (everything SBUF-resident), and the per-512-chunk
phases are fused into one continuous PE stream:
  for sc in 0..3:  V-proj(sc) -> K-proj+RoPE(sc) -> Q-proj+RoPE(sc)
                   -> attention(sc) -> o_proj(sc-1)
so the tensor engine never waits on DMA and the HAM clock stays warm.
The attention inner loop software-pipelines scores 2 tiles ahead of
P@V so the ACT exp latency is hidden.  PSUM budget: acc(2) + scores(3)
+ attn-out(2) + softmax-sum(1) = 8 banks.
"""
import os
import sys

if "/opt/trn_rl_repo" not in sys.path:
    sys.path.insert(0, "/opt/trn_rl_repo")

import numpy as np
import ml_dtypes

import concourse.bass as bass
import concourse.mybir as mybir
import concourse.tile as tile
from concourse import bacc
from concourse.bass_utils import run_bass_kernel_spmd
from contextlib import ExitStack

P = 128
B, S, HID, NH = 2, 2048, 2048, 16
HD = HID // NH              # 128
H = 4                       # heads per core
DPC = H * HD                # 512 dims per core
KO = HID // P               # 16 contraction chunks
SC = S // 512               # 4 seq chunks of 512
ST = S // P                 # 16 seq tiles of 128
SCALE = 1.0 / float(np.sqrt(HD))
LOOK = 2                    # attention pipeline lookahead (tiles)

f32 = mybir.dt.float32
bf16 = mybir.dt.bfloat16

_CACHED_NC = None


def build_nc():
    AF = mybir.ActivationFunctionType
    nc = bacc.Bacc(None, target_bir_lowering=False)

    xt = nc.declare_dram_parameter("xt", [P, KO, S], bf16, isOutput=False)
    wq = nc.declare_dram_parameter("wq", [H, P, KO, HD], bf16, isOutput=False)
    wk = nc.declare_dram_parameter("wk", [H, P, KO, HD], bf16, isOutput=False)
    wv = nc.declare_dram_parameter("wv", [P, KO, DPC], bf16, isOutput=False)
    wo = nc.declare_dram_parameter("wo", [P, H, HID], bf16, isOutput=False)
    cosf = nc.declare_dram_parameter("cosf", [P, S], f32, isOutput=False)
    sinf = nc.declare_dram_parameter("sinf", [P, S], f32, isOutput=False)
    bmask = nc.declare_dram_parameter("bmask", [P, 4, 512], bf16, isOutput=False)
    out_p = nc.declare_dram_parameter("out_p", [S, HID], f32, isOutput=True)

    out3 = out_p.rearrange("(st p) n -> p st n", p=P)

    with tile.TileContext(nc) as tc:
        with ExitStack() as top:
            const = top.enter_context(tc.tile_pool(name="const", bufs=1))
            wpool = top.enter_context(tc.tile_pool(name="wpool", bufs=1))
            kvpool = top.enter_context(tc.tile_pool(name="kv", bufs=1))
            xpool = top.enter_context(tc.tile_pool(name="xp", bufs=2))
            qpool = top.enter_context(tc.tile_pool(name="qp", bufs=2))
            aopool = top.enter_context(tc.tile_pool(name="ao", bufs=2))
            rtmp = top.enter_context(tc.tile_pool(name="rt", bufs=2))
            ppool = top.enter_context(tc.tile_pool(name="pp", bufs=6))
            ost = top.enter_context(tc.tile_pool(name="ost", bufs=4))
            stage = top.enter_context(tc.tile_pool(name="stage", bufs=2))
            # PSUM: exactly 8 banks
            acc = top.enter_context(tc.tile_pool(name="acc", bufs=2, space="PSUM"))
            sps = top.enter_context(tc.tile_pool(name="sps", bufs=3, space="PSUM"))
            obp = top.enter_context(tc.tile_pool(name="obp", bufs=2, space="PSUM"))
            smp = top.enter_context(tc.tile_pool(name="smp", bufs=1, space="PSUM"))

            # ---- static tiles ----
            wvs = wpool.tile([P, KO, DPC], bf16)
            wqs = wpool.tile([P, H, KO, HD], bf16)
            wks = wpool.tile([P, H, KO, HD], bf16)
            wot = wpool.tile([P, H, HID], bf16)
            cosT = const.tile([P, S], f32)
            sinT = const.tile([P, S], f32)
            bmt = const.tile([P, 4, 512], bf16)
            zb = const.tile([P, 1], f32)
            ones_col = const.tile([P, 1], bf16)
            kt = kvpool.tile([P, H, S], bf16)        # K^T, RoPE'd, all chunks
            vsb = kvpool.tile([P, ST, H, HD], bf16)  # V natural layout

            nc.vector.memset(zb[:], 0.0)
            nc.vector.memset(ones_col[:], 1.0)

            # ---- load order: wv first (first matmuls), then x0, then the
            # rest; split halves across the two HWDGE queues ----
            nc.sync.dma_start(wvs[:, :, 0:256], wv[:, :, 0:256])
            nc.scalar.dma_start(wvs[:, :, 256:512], wv[:, :, 256:512])
            xs_c = [xpool.tile([P, KO, 512], bf16, tag="xs", name=f"xs{c}")
                    for c in range(SC)]
            for ko in range(KO):
                eng = nc.sync if ko % 2 == 0 else nc.scalar
                eng.dma_start(xs_c[0][:, ko], xt[:, ko, 0:512])
            for h in range(H):
                nc.sync.dma_start(wks[:, h], wk[h])
                nc.scalar.dma_start(wqs[:, h], wq[h])
            nc.scalar.dma_start(cosT[:], cosf[:])
            nc.scalar.dma_start(sinT[:], sinf[:])
            nc.scalar.dma_start(bmt[:], bmask[:])
            nc.sync.dma_start(wot[:], wo[:])

            qt_c = [None] * SC
            aot_c = [None] * SC

            def emit_vproj(sc):
                for st4 in range(4):
                    st = sc * 4 + st4
                    ps = acc.tile([P, DPC], f32, tag="acc")
                    for ko in range(KO):
                        nc.tensor.matmul(
                            ps[:],
                            xs_c[sc][:, ko, st4 * P:(st4 + 1) * P],
                            wvs[:, ko],
                            start=(ko == 0),
                            stop=(ko == KO - 1),
                        )
                    nc.vector.tensor_copy(
                        vsb[:, st], ps.rearrange("p (h d) -> p h d", h=H)
                    )

            def emit_qkproj(sc, ws, dst, dst_sl):
                # dst[dst_sl(h)] <- RoPE(ws[h].T @ x_chunk) in bf16
                ssl = slice(sc * 512, (sc + 1) * 512)
                for h in range(H):
                    ps = acc.tile([P, 512], f32, tag="acc")
                    for ko in range(KO):
                        nc.tensor.matmul(
                            ps[:],
                            ws[:, h, ko],
                            xs_c[sc][:, ko],
                            start=(ko == 0),
                            stop=(ko == KO - 1),
                        )
                    # RoPE eviction: partition-shifted reads legal on PSUM;
                    # sinT is pre-signed (-sin rows 0:63, +sin rows 64:127)
                    t0 = rtmp.tile([P, 512], f32, tag="t0")
                    c0 = rtmp.tile([P, 512], f32, tag="c0")
                    nc.vector.tensor_mul(t0[0:64], ps[64:128], sinT[0:64, ssl])
                    nc.vector.tensor_mul(t0[64:128], ps[0:64], sinT[64:128, ssl])
                    nc.vector.tensor_mul(c0[:], ps[:], cosT[:, ssl])
                    nc.vector.tensor_add(dst[dst_sl(h)], c0[:], t0[:])

            def emit_attention(sc):
                nt = 4 * (sc + 1)
                qc = qt_c[sc]
                aot = aot_c[sc]
                t_order = list(range(4 * sc, nt)) + list(range(0, 4 * sc))
                for h in range(H):
                    ob = obp.tile([P, 512], f32, tag="ob")
                    sm = smp.tile([1, 512], f32, tag="sm")
                    pend = []

                    def flush_one():
                        pt, off, ti, t = pend.pop(0)
                        nc.tensor.matmul(
                            ob[:, off:512],
                            vsb[:, t, h],
                            pt[:, off:512],
                            start=(ti == 0),
                            stop=(ti == nt - 1),
                        )
                        nc.tensor.matmul(
                            sm[:, off:512],
                            ones_col[:],
                            pt[:, off:512],
                            start=(ti == 0),
                            stop=(ti == nt - 1),
                        )

                    for ti, t in enumerate(t_order):
                        r = t - 4 * sc
                        off = P * max(r, 0)
                        ps = sps.tile([P, 512], f32, tag="s")
                        nc.tensor.matmul(
                            ps[:, off:512],
                            kt[:, h, t * P:(t + 1) * P],
                            qc[:, h, off:512],
                            start=True,
                            stop=True,
                        )
                        pt = ppool.tile([P, 512], bf16, tag="pt")
                        nc.scalar.activation(
                            pt[:, off:512], ps[:, off:512], AF.Exp,
                            bias=zb[:], scale=SCALE,
                        )
                        if r >= 0:
                            nc.vector.tensor_mul(
                                pt[:, off:512], pt[:, off:512], bmt[:, r, off:512]
                            )
                        pend.append((pt, off, ti, t))
                        if len(pend) > LOOK:
                            flush_one()
                    while pend:
                        flush_one()
                    # normalize: rcp -> broadcast to 128 partitions -> one mul
                    rcp = stage.tile([1, 512], f32, tag="rcp")
                    nc.vector.reciprocal_approx_fast(rcp[:], sm[:])
                    bc = stage.tile([P, 512], f32, tag="bc")
                    nc.gpsimd.partition_broadcast(bc[:], rcp[:])
                    nc.vector.tensor_mul(aot[:, h], ob[:], bc[:])

            def emit_oproj(cc):
                aot = aot_c[cc]
                for st4 in range(4):
                    st = cc * 4 + st4
                    for nch in range(4):
                        pso = acc.tile([P, 512], f32, tag="acc")
                        for dc in range(H):
                            nc.tensor.matmul(
                                pso[:],
                                aot[:, dc, st4 * P:(st4 + 1) * P],
                                wot[:, dc, nch * 512:(nch + 1) * 512],
                                start=(dc == 0),
                                stop=(dc == H - 1),
                            )
                        ob2 = ost.tile([P, 512], f32, tag="ob2")
                        nc.vector.tensor_copy(ob2[:], pso[:])
                        nc.sync.dma_start(
                            out3[:, st, nch * 512:(nch + 1) * 512], ob2[:]
                        )

            for sc in range(SC):
                # prefetch next x chunk
                if sc + 1 < SC:
                    for ko in range(KO):
                        eng = nc.sync if ko % 2 == 0 else nc.scalar
                        eng.dma_start(
                            xs_c[sc + 1][:, ko],
                            xt[:, ko, (sc + 1) * 512:(sc + 2) * 512],
                        )
                qt_c[sc] = qpool.tile([P, H, 512], bf16, tag="qt", name=f"qt{sc}")
                aot_c[sc] = aopool.tile([P, H, 512], bf16, tag="aot", name=f"aot{sc}")
                emit_vproj(sc)
                emit_qkproj(sc, wks, kt,
                            lambda h, _s=sc: (slice(None), h, slice(_s * 512, (_s + 1) * 512)))
                emit_qkproj(sc, wqs, qt_c[sc],
                            lambda h: (slice(None), h, slice(None)))
                emit_attention(sc)
                if sc > 0:
                    emit_oproj(sc - 1)
            emit_oproj(SC - 1)

    nc.compile()
    return nc


def _host_prep(hidden_states, position_ids, Wq, Wk, Wv, Wo):
    """Build the 8 per-core input maps (bf16 operands)."""
    inv_freq = 1.0 / (10000.0 ** (np.arange(0, HD, 2, dtype=np.float32) / HD))
    t = np.arange(S, dtype=np.float32)
    freqs = np.outer(t, inv_freq).astype(np.float32)  # [S, 64]

    bm = np.empty((P, 4, 512), dtype=np.float32)
    i = np.arange(P)[:, None, None]
    r = np.arange(4)[None, :, None]
    j = np.arange(512)[None, None, :]
    bm[:] = np.where(i + P * r <= j, 1.0, 0.0)
    bm = bm.astype(ml_dtypes.bfloat16)

    in_maps = []
    per_batch = []
    for b in range(B):
        xT = np.ascontiguousarray(hidden_states[b].T)  # [HID, S]
        xt_sw = np.ascontiguousarray(
            xT.reshape(KO, P, S).transpose(1, 0, 2)
        ).astype(ml_dtypes.bfloat16)  # [P, KO, S]
        fp = freqs[position_ids[b]]  # [S, 64]
        ch = np.cos(fp).T            # [64, S]
        sh = np.sin(fp).T
        cosf = np.ascontiguousarray(np.concatenate([ch, ch], axis=0))   # [128, S]
        sinf = np.ascontiguousarray(np.concatenate([-sh, sh], axis=0))  # signed
        per_batch.append((xt_sw, cosf, sinf))

    for core in range(8):
        b, hg = core // 4, core % 4
        sl = slice(hg * DPC, (hg + 1) * DPC)
        xt_sw, cosf, sinf = per_batch[b]
        wq_sw = np.ascontiguousarray(
            Wq[sl].T.reshape(KO, P, H, HD).transpose(2, 1, 0, 3)
        ).astype(ml_dtypes.bfloat16)  # [H, P, KO, HD]
        wk_sw = np.ascontiguousarray(
            Wk[sl].T.reshape(KO, P, H, HD).transpose(2, 1, 0, 3)
        ).astype(ml_dtypes.bfloat16)
        wv_sw = np.ascontiguousarray(
            Wv[sl].T.reshape(KO, P, DPC).transpose(1, 0, 2)
        ).astype(ml_dtypes.bfloat16)  # [P, KO, DPC]
        wo_sw = np.ascontiguousarray(
            Wo[:, sl].T.reshape(H, HD, HID).transpose(1, 0, 2)
        ).astype(ml_dtypes.bfloat16)  # [P, H, HID]
        in_maps.append({
            "xt": xt_sw, "wq": wq_sw, "wk": wk_sw, "wv": wv_sw, "wo": wo_sw,
            "cosf": cosf, "sinf": sinf, "bmask": bm,
        })
    return in_maps


def kernel(hidden_states, attention_mask, position_ids, Wq, Wk, Wv, Wo,
           _trace=False, _trace_kwargs=None):
    global _CACHED_NC
    hidden_states = np.asarray(hidden_states, dtype=np.float32)
    position_ids = np.asarray(position_ids)
    Wq, Wk, Wv, Wo = (np.asarray(w, dtype=np.float32) for w in (Wq, Wk, Wv, Wo))

    if _CACHED_NC is None:
        _CACHED_NC = build_nc()
    nc = _CACHED_NC

    in_maps = _host_prep(hidden_states, position_ids, Wq, Wk, Wv, Wo)
    res = run_bass_kernel_spmd(
        nc, in_maps, list(range(8)), trace=_trace, **(_trace_kwargs or {})
    )

    out = np.empty((B, S, HID), dtype=np.float32)
    for b in range(B):
        acc = res.results[b * 4]["out_p"].astype(np.float32)
        for hg in range(1, 4):
            acc = acc + res.results[b * 4 + hg]["out_p"]
        out[b] = acc
    if _trace:
        return out, res
    return out


# revision 6
# speedup vs baseline: 1.4549x; 1.0394x over previous
"""TRN2 Bass kernel for causal multi-head attention with RoPE (v3).

Problem: B=2, S=2048, HID=2048, NH=16, HD=128 (fp32 reference).
Sharding: 8 cores = 2 (batch) x 4 (head-groups of 4 heads).
Each core computes q/k/v projections for its 4 heads (column-parallel),
RoPE, causal attention, and a row-parallel partial o_proj; the host sums
the 4 partials per batch.

v3 design: all matmul operands bf16 (PSUM accumulates fp32), everything
SBUF-resident (no DRAM spill), and one continuous PE stream where the
attention tiles of chunk c are interleaved with "filler" matmuls --
o_proj(c-1) and the q/k/v projections of chunk c+1.  The filler PE work
hides the ACT exp latency of the attention softmax, so neither engine
gates: the kernel runs at the tensor-engine roofline end to end and the
HAM clock never re-throttles.  PSUM: acc(2) + scores(3) + attn-out(2) +
softmax-sum(1) = 8 banks.
"""
import os
import sys

if "/opt/trn_rl_repo" not in sys.path:
    sys.path.insert(0, "/opt/trn_rl_repo")

import numpy as np
import ml_dtypes

import concourse.bass as bass
import concourse.mybir as mybir
import concourse.tile as tile
from concourse import bacc
from concourse.bass_utils import run_bass_kernel_spmd
from contextlib import ExitStack

P = 128
B, S, HID, NH = 2, 2048, 2048, 16
HD = HID // NH              # 128
H = 4                       # heads per core
DPC = H * HD                # 512 dims per core
KO = HID // P               # 16 contraction chunks
SC = S // 512               # 4 seq chunks of 512
ST = S // P                 # 16 seq tiles of 128
SCALE = 1.0 / float(np.sqrt(HD))
LOOK = 2                    # attention pipeline lookahead (tiles)

f32 = mybir.dt.float32
bf16 = mybir.dt.bfloat16

_CACHED_NC = None


def build_nc():
    AF = mybir.ActivationFunctionType
    nc = bacc.Bacc(None, target_bir_lowering=False)

    xt = nc.declare_dram_parameter("xt", [P, KO, S], bf16, isOutput=False)
    wq = nc.declare_dram_parameter("wq", [H, P, KO, HD], bf16, isOutput=False)
    wk = nc.declare_dram_parameter("wk", [H, P, KO, HD], bf16, isOutput=False)
    wv = nc.declare_dram_parameter("wv", [P, KO, DPC], bf16, isOutput=False)
    wo = nc.declare_dram_parameter("wo", [P, H, HID], bf16, isOutput=False)
    cosf = nc.declare_dram_parameter("cosf", [P, S], f32, isOutput=False)
    sinf = nc.declare_dram_parameter("sinf", [P, S], f32, isOutput=False)
    bmask = nc.declare_dram_parameter("bmask", [P, 4, 512], bf16, isOutput=False)
    out_p = nc.declare_dram_parameter("out_p", [S, HID], f32, isOutput=True)

    out3 = out_p.rearrange("(st p) n -> p st n", p=P)

    with tile.TileContext(nc) as tc:
        with ExitStack() as top:
            const = top.enter_context(tc.tile_pool(name="const", bufs=1))
            wpool = top.enter_context(tc.tile_pool(name="wpool", bufs=1))
            kvpool = top.enter_context(tc.tile_pool(name="kv", bufs=1))
            xpool = top.enter_context(tc.tile_pool(name="xp", bufs=2))
            qpool = top.enter_context(tc.tile_pool(name="qp", bufs=2))
            aopool = top.enter_context(tc.tile_pool(name="ao", bufs=2))
            rtmp = top.enter_context(tc.tile_pool(name="rt", bufs=2))
            ppool = top.enter_context(tc.tile_pool(name="pp", bufs=6))
            ost = top.enter_context(tc.tile_pool(name="ost", bufs=4))
            stage = top.enter_context(tc.tile_pool(name="stage", bufs=2))
            # PSUM: exactly 8 banks
            acc = top.enter_context(tc.tile_pool(name="acc", bufs=2, space="PSUM"))
            sps = top.enter_context(tc.tile_pool(name="sps", bufs=3, space="PSUM"))
            obp = top.enter_context(tc.tile_pool(name="obp", bufs=2, space="PSUM"))
            smp = top.enter_context(tc.tile_pool(name="smp", bufs=1, space="PSUM"))

            # ---- static tiles ----
            wvs = wpool.tile([P, KO, DPC], bf16)
            wqs = wpool.tile([P, H, KO, HD], bf16)
            wks = wpool.tile([P, H, KO, HD], bf16)
            wot = wpool.tile([P, H, HID], bf16)
            cosT = const.tile([P, S], f32)
            sinT = const.tile([P, S], f32)
            bmt = const.tile([P, 4, 512], bf16)
            zb = const.tile([P, 1], f32)
            ones_col = const.tile([P, 1], bf16)
            kt = kvpool.tile([P, H, S], bf16)        # K^T, RoPE'd, all chunks
            vsb = kvpool.tile([P, ST, H, HD], bf16)  # V natural layout

            nc.vector.memset(zb[:], 0.0)
            nc.vector.memset(ones_col[:], 1.0)

            # ---- load order tuned for the ko-pipelined chunk-0 V-proj:
            # wv ko-quarters and per-ko x0 slices interleaved on both queues,
            # then wk (sync) / cos+sin+wq (scalar), wot last ----
            xs_c = [xpool.tile([P, KO, 512], bf16, tag="xs", name=f"xs{c}")
                    for c in range(SC)]
            nc.sync.dma_start(wvs[:, 0:4], wv[:, 0:4])
            nc.scalar.dma_start(wvs[:, 4:8], wv[:, 4:8])
            nc.sync.dma_start(xs_c[0][:, 0], xt[:, 0, 0:512])
            nc.scalar.dma_start(xs_c[0][:, 1], xt[:, 1, 0:512])
            nc.sync.dma_start(wvs[:, 8:12], wv[:, 8:12])
            nc.scalar.dma_start(wvs[:, 12:16], wv[:, 12:16])
            for ko in range(2, KO):
                eng = nc.sync if ko % 2 == 0 else nc.scalar
                eng.dma_start(xs_c[0][:, ko], xt[:, ko, 0:512])
            for h in range(H):
                nc.sync.dma_start(wks[:, h], wk[h])
            nc.scalar.dma_start(cosT[:], cosf[:])
            nc.scalar.dma_start(sinT[:], sinf[:])
            for h in range(H):
                nc.scalar.dma_start(wqs[:, h], wq[h])
            nc.scalar.dma_start(bmt[:], bmask[:])
            nc.sync.dma_start(wot[:], wo[:])

            qt_c = [None] * SC
            aot_c = [None] * SC

            # ---------- filler item builders (each item: emit ~1 matmul) ----
            def vproj_items(sc):
                items = []
                box = {}
                for st4 in range(4):
                    st = sc * 4 + st4
                    for ko in range(KO):
                        def mk(st=st, st4=st4, ko=ko):
                            if ko == 0:
                                box['ps'] = acc.tile([P, DPC], f32, tag="acc", name="vps")
                            nc.tensor.matmul(
                                box['ps'][:],
                                xs_c[sc][:, ko, st4 * P:(st4 + 1) * P],
                                wvs[:, ko],
                                start=(ko == 0),
                                stop=(ko == KO - 1),
                            )
                            if ko == KO - 1:
                                nc.vector.tensor_copy(
                                    vsb[:, st],
                                    box['ps'].rearrange("p (h d) -> p h d", h=H),
                                )
                        items.append(mk)
                return items

            def qkproj_items(sc, ws, dst, dst_sl):
                # dst[dst_sl(h)] <- RoPE(ws[h].T @ x_chunk) in bf16
                ssl = slice(sc * 512, (sc + 1) * 512)
                items = []
                box = {}
                for h in range(H):
                    for ko in range(KO):
                        def mk(h=h, ko=ko):
                            if ko == 0:
                                box['ps'] = acc.tile([P, 512], f32, tag="acc", name="qkps")
                            ps = box['ps']
                            nc.tensor.matmul(
                                ps[:],
                                ws[:, h, ko],
                                xs_c[sc][:, ko],
                                start=(ko == 0),
                                stop=(ko == KO - 1),
                            )
                            if ko == KO - 1:
                                # RoPE eviction; sinT pre-signed (-sin top)
                                t0 = rtmp.tile([P, 512], f32, tag="t0", name="t0")
                                c0 = rtmp.tile([P, 512], f32, tag="c0", name="c0")
                                nc.vector.tensor_mul(
                                    t0[0:64], ps[64:128], sinT[0:64, ssl])
                                nc.vector.tensor_mul(
                                    t0[64:128], ps[0:64], sinT[64:128, ssl])
                                nc.vector.tensor_mul(c0[:], ps[:], cosT[:, ssl])
                                nc.vector.tensor_add(dst[dst_sl(h)], c0[:], t0[:])
                        items.append(mk)
                return items

            def oproj_items(cc):
                aot = aot_c[cc]
                items = []
                box = {}
                for st4 in range(4):
                    st = cc * 4 + st4
                    for nch in range(4):
                        for dc in range(H):
                            def mk(st=st, st4=st4, nch=nch, dc=dc):
                                if dc == 0:
                                    box['ps'] = acc.tile([P, 512], f32, tag="acc", name="ops")
                                pso = box['ps']
                                nc.tensor.matmul(
                                    pso[:],
                                    aot[:, dc, st4 * P:(st4 + 1) * P],
                                    wot[:, dc, nch * 512:(nch + 1) * 512],
                                    start=(dc == 0),
                                    stop=(dc == H - 1),
                                )
                                if dc == H - 1:
                                    ob2 = ost.tile([P, 512], f32, tag="ob2", name="ob2")
                                    nc.vector.tensor_copy(ob2[:], pso[:])
                                    nc.sync.dma_start(
                                        out3[:, st, nch * 512:(nch + 1) * 512],
                                        ob2[:],
                                    )
                            items.append(mk)
                return items

            # ---------- attention ----------
            def attention_tiles(sc):
                """Return a list of per-tile emitters; each emits scores+exp
                (+mask), appends to a pend queue, and flushes P@V/sum for the
                tile LOOK back (plus the normalize chain at head end)."""
                nt = 4 * (sc + 1)
                qc = qt_c[sc]
                aot = aot_c[sc]
                t_order = list(range(4 * sc, nt)) + list(range(0, 4 * sc))
                pend = []

                def flush_one():
                    pt, off, ti, t, h, ob, sm = pend.pop(0)
                    nc.tensor.matmul(
                        ob[:, off:512], vsb[:, t, h], pt[:, off:512],
                        start=(ti == 0), stop=(ti == nt - 1),
                    )
                    nc.tensor.matmul(
                        sm[:, off:512], ones_col[:], pt[:, off:512],
                        start=(ti == 0), stop=(ti == nt - 1),
                    )
                    if ti == nt - 1:
                        rcp = stage.tile([1, 512], f32, tag="rcp", name="rcp")
                        nc.vector.reciprocal_approx_fast(rcp[:], sm[:])
                        bc = stage.tile([P, 512], f32, tag="bc", name="bc")
                        nc.gpsimd.partition_broadcast(bc[:], rcp[:])
                        nc.vector.tensor_mul(aot[:, h], ob[:], bc[:])

                items = []
                box = {}
                for h in range(H):
                    for ti, t in enumerate(t_order):
                        def mk(h=h, ti=ti, t=t):
                            if ti == 0:
                                box['ob'] = obp.tile([P, 512], f32, tag="ob", name="ob")
                                box['sm'] = smp.tile([1, 512], f32, tag="sm", name="sm")
                            r = t - 4 * sc
                            off = P * max(r, 0)
                            ps = sps.tile([P, 512], f32, tag="s", name="sco")
                            nc.tensor.matmul(
                                ps[:, off:512],
                                kt[:, h, t * P:(t + 1) * P],
                                qc[:, h, off:512],
                                start=True, stop=True,
                            )
                            pt = ppool.tile([P, 512], bf16, tag="pt", name="pt")
                            nc.scalar.activation(
                                pt[:, off:512], ps[:, off:512], AF.Exp,
                                bias=zb[:], scale=SCALE,
                            )
                            if r >= 0:
                                nc.vector.tensor_mul(
                                    pt[:, off:512], pt[:, off:512],
                                    bmt[:, r, off:512],
                                )
                            pend.append(
                                (pt, off, ti, t, h, box['ob'], box['sm']))
                            if len(pend) > LOOK:
                                flush_one()
                        items.append(mk)

                def drain():
                    while pend:
                        flush_one()
                return items, drain

            # ---------- main schedule ----------
            for sc in range(SC):
                # prefetch next x chunk on both queues
                if sc + 1 < SC:
                    for ko in range(KO):
                        eng = nc.sync if ko % 2 == 0 else nc.scalar
                        eng.dma_start(
                            xs_c[sc + 1][:, ko],
                            xt[:, ko, (sc + 1) * 512:(sc + 2) * 512],
                        )
                aot_c[sc] = aopool.tile([P, H, 512], bf16, tag="aot",
                                        name=f"aot{sc}")
                if sc == 0:
                    # no preceding attention to interleave with: emit densely
                    qt_c[0] = qpool.tile([P, H, 512], bf16, tag="qt", name="qt0")
                    for it in vproj_items(0):
                        it()
                    for it in qkproj_items(
                            0, wks, kt,
                            lambda h: (slice(None), h, slice(0, 512))):
                        it()
                    for it in qkproj_items(
                            0, wqs, qt_c[0],
                            lambda h: (slice(None), h, slice(None))):
                        it()

                # filler: o_proj(sc-1) first (no DMA dependency), then
                # projections of chunk sc+1
                filler = []
                if sc > 0:
                    filler += oproj_items(sc - 1)
                if sc + 1 < SC:
                    nsc = sc + 1
                    qt_c[nsc] = qpool.tile([P, H, 512], bf16, tag="qt",
                                           name=f"qt{nsc}")
                    filler += vproj_items(nsc)
                    filler += qkproj_items(
                        nsc, wks, kt,
                        lambda h, _s=nsc: (slice(None), h,
                                           slice(_s * 512, (_s + 1) * 512)))
                    filler += qkproj_items(
                        nsc, wqs, qt_c[nsc],
                        lambda h: (slice(None), h, slice(None)))

                tiles, drain = attention_tiles(sc)
                step = len(filler) / len(tiles)
                fi_target = 0.0
                fi = 0
                for it in tiles:
                    it()
                    fi_target += step
                    while fi < int(fi_target):
                        filler[fi]()
                        fi += 1
                drain()
                while fi < len(filler):
                    filler[fi]()
                    fi += 1

            for it in oproj_items(SC - 1):
                it()

    nc.compile()
    return nc


def _host_prep(hidden_states, position_ids, Wq, Wk, Wv, Wo):
    """Build the 8 per-core input maps (bf16 operands)."""
    inv_freq = 1.0 / (10000.0 ** (np.arange(0, HD, 2, dtype=np.float32) / HD))
    t = np.arange(S, dtype=np.float32)
    freqs = np.outer(t, inv_freq).astype(np.float32)  # [S, 64]

    bm = np.empty((P, 4, 512), dtype=np.float32)
    i = np.arange(P)[:, None, None]
    r = np.arange(4)[None, :, None]
    j = np.arange(512)[None, None, :]
    bm[:] = np.where(i + P * r <= j, 1.0, 0.0)
    bm = bm.astype(ml_dtypes.bfloat16)

    in_maps = []
    per_batch = []
    for b in range(B):
        xT = np.ascontiguousarray(hidden_states[b].T)  # [HID, S]
        xt_sw = np.ascontiguousarray(
            xT.reshape(KO, P, S).transpose(1, 0, 2)
        ).astype(ml_dtypes.bfloat16)  # [P, KO, S]
        fp = freqs[position_ids[b]]  # [S, 64]
        ch = np.cos(fp).T            # [64, S]
        sh = np.sin(fp).T
        cosf = np.ascontiguousarray(np.concatenate([ch, ch], axis=0))   # [128, S]
        sinf = np.ascontiguousarray(np.concatenate([-sh, sh], axis=0))  # signed
        per_batch.append((xt_sw, cosf, sinf))

    for core in range(8):
        b, hg = core // 4, core % 4
        sl = slice(hg * DPC, (hg + 1) * DPC)
        xt_sw, cosf, sinf = per_batch[b]
        wq_sw = np.ascontiguousarray(
            Wq[sl].T.reshape(KO, P, H, HD).transpose(2, 1, 0, 3)
        ).astype(ml_dtypes.bfloat16)  # [H, P, KO, HD]
        wk_sw = np.ascontiguousarray(
            Wk[sl].T.reshape(KO, P, H, HD).transpose(2, 1, 0, 3)
        ).astype(ml_dtypes.bfloat16)
        wv_sw = np.ascontiguousarray(
            Wv[sl].T.reshape(KO, P, DPC).transpose(1, 0, 2)
        ).astype(ml_dtypes.bfloat16)  # [P, KO, DPC]
        wo_sw = np.ascontiguousarray(
            Wo[:, sl].T.reshape(H, HD, HID).transpose(1, 0, 2)
        ).astype(ml_dtypes.bfloat16)  # [P, H, HID]
        in_maps.append({
            "xt": xt_sw, "wq": wq_sw, "wk": wk_sw, "wv": wv_sw, "wo": wo_sw,
            "cosf": cosf, "sinf": sinf, "bmask": bm,
        })
    return in_maps


def kernel(hidden_states, attention_mask, position_ids, Wq, Wk, Wv, Wo,
           _trace=False, _trace_kwargs=None):
    global _CACHED_NC
    hidden_states = np.asarray(hidden_states, dtype=np.float32)
    position_ids = np.asarray(position_ids)
    Wq, Wk, Wv, Wo = (np.asarray(w, dtype=np.float32) for w in (Wq, Wk, Wv, Wo))

    if _CACHED_NC is None:
        _CACHED_NC = build_nc()
    nc = _CACHED_NC

    in_maps = _host_prep(hidden_states, position_ids, Wq, Wk, Wv, Wo)
    res = run_bass_kernel_spmd(
        nc, in_maps, list(range(8)), trace=_trace, **(_trace_kwargs or {})
    )

    out = np.empty((B, S, HID), dtype=np.float32)
    for b in range(B):
        acc = res.results[b * 4]["out_p"].astype(np.float32)
        for hg in range(1, 4):
            acc = acc + res.results[b * 4 + hg]["out_p"]
        out[b] = acc
    if _trace:
        return out, res
    return out
